# revision 16
# baseline (speedup 1.0000x reference)
"""Trainium2 Bass kernel for the Hebbian fast-weight memory module.

Reference computation (B=256 batches, T=16 steps, M=256):
    step t:  p2 = learn * relu6(learn2*x_t + A @ x_t)
             A  = (1-decay)*A + outer(x_t, p2)
    output:  relu6(A_final @ x_query)

Key identity (g = 1-decay, Phi_s = relu6(z_s), z_s = learn2*x_s + y_s):
    y_t[i] = g^t (A_init@x_t)[i] + sum_{s<t} g^{t-1-s} learn (Phi_s . x_t) x_s[i]
    out[i] = relu6(g^16 (A_init@x_q)[i] + sum_s g^{15-s} learn (Phi_s . x_q) x_s[i])
A is never materialized. For the fast path we additionally require
A_init == 0 and xs, x_query >= 0 (checked at runtime): then z >= 0 and
relu6(z) = min(z, 6) folds into the history dot products.

Fast-path layout per core (bpc=32 batches):
  partitions p = s4*32 + b  (s4 in [0,4), b in [0,32)), history step s = 4k+s4
  z-history lives IN PSUM: z_s at PSUM bank (s//4), partitions (s%4)*32+b,
  written directly by the accumulating matmuls (no per-step copy-out).
  Completed history tiles get one lazy ACT copy to SBUF (bf16), 3 total.

  Per step t: for each history tile k: a scalar_tensor_tensor on DVE/Pool
  computes cw_k[p] = sum_m min(z,6)*x_t (relu6 folded); ACT/DVE builds
  selcw_k = g^t*onehot(b)*cw_k; PE accumulates selcw_k.T @ xhw_k into
  z_t's PSUM slot. learn2*x_t enters via a tiny constant matmul (eyeL2).

Sharding: batch 256 -> 8 cores x 32 batches (pure data parallel).
The general path (A_init != 0 or negative inputs) uses the original
baseline program with host-precomputed additive terms.
"""

import os
import sys

for _p in ("/opt/pypackages", "/opt/trn_rl_repo"):
    if _p not in sys.path:
        sys.path.insert(0, _p)

import numpy as np

B, T, M = 256, 16, 256
NCORES = 8
BPC = B // NCORES  # 32 batches per core
NSTILE = 4         # history tiles; each holds 4 steps x 32 batches

_COMPILED = {}


def _dve_old(t, k):
    """True if the old-tile dot (t, k) runs on DVE instead of Pool."""
    return (t * 3 + k) % 4 == 0


def _build_program_fast():
    import concourse.bacc as bacc
    import concourse.mybir as mybir
    from concourse.tile import TileContext

    f32 = mybir.dt.float32
    bf16 = mybir.dt.bfloat16
    Alu = mybir.AluOpType
    Act = mybir.ActivationFunctionType

    nc = bacc.Bacc(target_bir_lowering=False)

    NQ = T + 1  # 16 steps + query
    xb_d = nc.dram_tensor("xb", [128, NQ * M], bf16, kind="ExternalInput")
    xhw_d = nc.dram_tensor("xhw", [128, NSTILE * M], bf16, kind="ExternalInput")
    selw_d = nc.dram_tensor("selw", [128, NQ * BPC], bf16, kind="ExternalInput")
    eyel2_d = nc.dram_tensor("eyel2", [BPC, BPC], bf16, kind="ExternalInput")
    out_d = nc.dram_tensor("out", [BPC, M], f32, kind="ExternalOutput")

    with TileContext(nc) as tc:
        with (
            tc.tile_pool(name="persist", bufs=1) as pp,
            tc.tile_pool(name="work", bufs=28) as wp,
            tc.tile_pool(name="psum", bufs=1, space="PSUM") as psp,
        ):
            xb_all = pp.tile([128, NQ * M], bf16, tag="xb", name="xb_sb")
            xb = [xb_all[:, t * M:(t + 1) * M] for t in range(NQ)]
            xhw_all = pp.tile([128, NSTILE * M], bf16, tag="xhw", name="xhw_sb")
            xhw = [xhw_all[:, k * M:(k + 1) * M] for k in range(NSTILE)]
            selw_all = pp.tile([128, NQ * BPC], bf16, tag="selw", name="selw_sb")
            selw = [selw_all[:, t * BPC:(t + 1) * BPC] for t in range(NQ)]
            eyel2 = pp.tile([BPC, BPC], bf16, tag="eyel2", name="eyel2_sb")
            junk_d = pp.tile([128, M], bf16, tag="junkd", name="junk_dve")
            junk_p = pp.tile([128, M], bf16, tag="junkp", name="junk_pool")
            # z history in PSUM: bank k holds steps 4k..4k+3 (k=4: query y)
            zb = [psp.tile([128, M], f32, tag=f"zb{k}", name=f"zb{k}")
                  for k in range(NSTILE + 1)]
            zjunk = psp.tile([BPC, M], f32, tag="zjunk", name="zjunk")

            # --- input DMAs, split across queues; earliest-needed first ---
            # All input DMAs ride the SP (sync) HWDGE queue: the SP engine
            # is idle so descriptors stream immediately; the gpsimd queue is
            # SWDGE (software-serial) and the scalar engine is busy with
            # selcw activations all kernel.
            XS = 5 * M
            nc.sync.dma_start(out=eyel2[:], in_=eyel2_d[:, :])
            nc.sync.dma_start(out=selw_all[:, :6 * BPC],
                              in_=selw_d[:, :6 * BPC])
            nc.sync.dma_start(out=xb_all[:, :2 * M], in_=xb_d[:, :2 * M])
            nc.scalar.dma_start(out=xhw_all[:], in_=xhw_d[:, :])
            nc.scalar.dma_start(out=selw_all[:, 6 * BPC:],
                                in_=selw_d[:, 6 * BPC:])
            nc.sync.dma_start(out=xb_all[:, 2 * M:XS],
                              in_=xb_d[:, 2 * M:XS])
            nc.sync.dma_start(out=xb_all[:, T * M:], in_=xb_d[:, T * M:])
            nc.sync.dma_start(out=xb_all[:, XS:T * M],
                              in_=xb_d[:, XS:T * M])

            def zslot(s):
                q = s % 4
                return zb[s // 4][q * BPC:(q + 1) * BPC, :]

            def hot_of(t):
                return (t - 1) // 4

            # ---- software-pipelined schedule for old-tile work ----
            # item (tp, k): dot of completed tile k against query x_tp.
            # Legal once ph[k] is copied (window 4k+4) and xb chunk holding
            # tp has landed; must be emitted no later than window tp-1.
            items = []
            for tp in range(2, NQ):
                for k in range(hot_of(tp)):
                    items.append((tp, k))

            def legal_w(tp, k):
                w = 4 * k + 4
                if tp > 4:
                    w = max(w, 3)
                if tp > 9:
                    w = max(w, 5)
                return max(w, 2)

            sched = {w: [] for w in range(1, NQ)}
            loads = {w: 0 for w in range(1, NQ)}
            for tp, k in sorted(items, key=lambda it: legal_w(*it)):
                w = legal_w(tp, k)
                cap = lambda x: 1 if x <= 4 else (2 if x <= 7 else 3)
                late = max(legal_w(tp, k), tp - 2)
                while loads[w] >= cap(w) and w < late:
                    w += 1
                w = min(w, late)
                sched[w].append((tp, k))
                loads[w] += 1

            selcw_of = {}   # (tp, k) -> selcw tile
            started = set()  # PSUM slots whose start mm has been emitted

            def emit_old_dot(tp, k):
                cw = wp.tile([128, 1], f32, tag="cw", name="cw")
                nc.vector.scalar_tensor_tensor(
                    out=junk_d[:], in0=zb[k][:], scalar=6.0, in1=xb[tp],
                    op0=Alu.min, op1=Alu.mult, accum_out=cw[:],
                )
                selcw = wp.tile([128, BPC], bf16, tag="selcw", name="selcw")
                nc.scalar.activation(
                    out=selcw[:], in_=selw[tp][:], func=Act.Copy,
                    scale=cw[:],
                )
                selcw_of[(tp, k)] = selcw

            def dst_of(t):
                return zslot(t) if t < T else zb[NSTILE][0:BPC, :]

            def tpos_of(t):
                return (0, (t % 4) * BPC) if t < T else (0, 0)

            def emit_start_and_old_mms(t):
                # eyeL2 start mm (z = learn2*x_t + ...); query has no
                # learn2 term, so its first old mm carries start=True
                if t < T:
                    nc.tensor.matmul(dst_of(t), eyel2[:], xb[t][0:BPC, :],
                                     start=True, stop=False,
                                     tile_position=tpos_of(t))
                    started.add(t)
                for k in range(hot_of(t)):
                    nc.tensor.matmul(
                        dst_of(t), selcw_of[(t, k)][:], xhw[k],
                        start=(t not in started), stop=False,
                        tile_position=tpos_of(t),
                    )
                    started.add(t)

            # t=0: z_0 = learn2 * x_0
            nc.tensor.matmul(zslot(0), eyel2[:], xb[0][0:BPC, :],
                             start=True, stop=True, tile_position=(0, 0))
            emit_start_and_old_mms(1)

            for t in range(1, NQ):
                is_q = (t == T)
                hot = hot_of(t)
                nrow = ((t - 1) % 4) + 1
                np_ = nrow * BPC

                # old-tile dots first: they depend only on completed PSUM
                # banks, so the DVE runs them while waiting for z_{t-1}
                for tp, k in sched.get(t, []):
                    emit_old_dot(tp, k)

                # --- hot chain: dot from PSUM, selcw on DVE, stop mm ---
                cw = wp.tile([128, 1], f32, tag="cw", name="cw")
                nc.vector.scalar_tensor_tensor(
                    out=junk_d[0:np_, :], in0=zb[hot][0:np_, :], scalar=6.0,
                    in1=xb[t][0:np_, :], op0=Alu.min, op1=Alu.mult,
                    accum_out=cw[0:np_, :],
                )
                selcw = wp.tile([128, BPC], bf16, tag="selcw", name="selcw")
                nc.vector.tensor_scalar(
                    out=selcw[0:np_, :], in0=selw[t][0:np_, :],
                    scalar1=cw[0:np_, :], scalar2=None, op0=Alu.mult,
                )
                nc.tensor.matmul(
                    dst_of(t), selcw[0:np_, :], xhw[hot][0:np_, :],
                    start=(t not in started), stop=True,
                    tile_position=tpos_of(t),
                )

                if t + 1 < NQ:
                    emit_start_and_old_mms(t + 1)

            res = wp.tile([BPC, M], f32, tag="res", name="res")
            nc.vector.tensor_scalar(
                out=res[:], in0=zb[NSTILE][0:BPC, :],
                scalar1=0.0, scalar2=6.0, op0=Alu.max, op1=Alu.min,
            )
            nc.sync.dma_start(out=out_d[:, :], in_=res[:])

    nc.finalize()
    return nc


def _build_program_general(dots_dtype):
    """Baseline program: general path (A_init != 0 or negative inputs)."""
    import concourse.bacc as bacc
    import concourse.mybir as mybir
    from concourse.tile import TileContext

    f32 = mybir.dt.float32
    bf16 = mybir.dt.bfloat16
    Alu = mybir.AluOpType

    nc = bacc.Bacc(target_bir_lowering=False)

    xb_d = nc.dram_tensor("xb", [128, (T + 1) * M], dots_dtype,
                          kind="ExternalInput")
    xhw_d = nc.dram_tensor("xhw", [128, NSTILE * M], bf16, kind="ExternalInput")
    selw_d = nc.dram_tensor("selw", [128, (T + 1) * BPC], bf16,
                            kind="ExternalInput")
    add_d = nc.dram_tensor("addt", [BPC, (T + 1) * M], f32,
                           kind="ExternalInput")
    out_d = nc.dram_tensor("out", [BPC, M], f32, kind="ExternalOutput")

    with TileContext(nc) as tc:
        with (
            tc.tile_pool(name="persist", bufs=1) as pp,
            tc.tile_pool(name="work", bufs=8) as wp,
            tc.tile_pool(name="psum", bufs=6, space="PSUM") as psp,
        ):
            xb_all = pp.tile([128, (T + 1) * M], dots_dtype, tag="xb",
                             name="xb_sb")
            xb = [xb_all[:, t * M:(t + 1) * M] for t in range(T + 1)]
            xhw_all = pp.tile([128, NSTILE * M], bf16, tag="xhw", name="xhw_sb")
            xhw = [xhw_all[:, k * M:(k + 1) * M] for k in range(NSTILE)]
            selw_all = pp.tile([128, (T + 1) * BPC], bf16, tag="selw",
                               name="selw_sb")
            selw = [selw_all[:, t * BPC:(t + 1) * BPC] for t in range(T + 1)]
            addt_all = pp.tile([BPC, (T + 1) * M], f32, tag="addt",
                               name="addt_sb")
            addt = [addt_all[:, t * M:(t + 1) * M] for t in range(T + 1)]
            ph = [pp.tile([128, M], dots_dtype, tag=f"ph{k}", name=f"ph{k}")
                  for k in range(NSTILE)]

            XB_SPLIT = 5 * M
            nc.scalar.dma_start(out=xb_all[:, :XB_SPLIT],
                                in_=xb_d[:, :XB_SPLIT])
            nc.scalar.dma_start(out=addt_all[:], in_=add_d[:, :])
            nc.sync.dma_start(out=selw_all[:], in_=selw_d[:, :])
            nc.sync.dma_start(out=xhw_all[:], in_=xhw_d[:, :])
            nc.sync.dma_start(out=xb_all[:, XB_SPLIT:], in_=xb_d[:, XB_SPLIT:])

            for k in range(NSTILE):
                nc.vector.memset(ph[k][:], 0.0)

            def step_y(t):
                y_ps = psp.tile([BPC, M], f32, tag="y", name="y")
                hot = min(max(t - 1, 0) // 4, NSTILE - 1)
                korder = list(range(hot + 1))
                for i, k in enumerate(korder):
                    junk = wp.tile([128, 1], f32, tag="junk", name="junk")
                    cw = wp.tile([128, 1], f32, tag="cw", name="cw")
                    nc.vector.scalar_tensor_tensor(
                        out=junk.broadcast_to((128, M)),
                        in0=ph[k][:],
                        scalar=1.0,
                        in1=xb[t],
                        op0=Alu.bypass,
                        op1=Alu.mult,
                        accum_out=cw[:],
                    )
                    selcw = wp.tile([128, BPC], bf16, tag="selcw",
                                    name="selcw")
                    nc.vector.tensor_scalar(
                        out=selcw[:], in0=selw[t], scalar1=cw[:],
                        scalar2=None, op0=Alu.mult,
                    )
                    nc.tensor.matmul(
                        y_ps[:], selcw[:], xhw[k],
                        start=(i == 0),
                        stop=(i == len(korder) - 1),
                    )
                return y_ps

            nc.vector.tensor_scalar(
                out=ph[0][0:BPC, :], in0=addt[0],
                scalar1=0.0, scalar2=6.0, op0=Alu.max, op1=Alu.min,
            )
            for t in range(1, T + 1):
                y_ps = step_y(t)
                if t < T:
                    k, s4 = t // 4, t % 4
                    dst = ph[k][s4 * BPC:(s4 + 1) * BPC, :]
                    zt = wp.tile([BPC, M], f32, tag="z", name="z")
                    nc.vector.tensor_add(out=zt[:], in0=y_ps[:],
                                         in1=addt[t])
                    nc.vector.tensor_scalar(
                        out=dst, in0=zt[:],
                        scalar1=0.0, scalar2=6.0,
                        op0=Alu.max, op1=Alu.min,
                    )
                else:
                    res = wp.tile([BPC, M], f32, tag="res", name="res")
                    z = wp.tile([BPC, M], f32, tag="z", name="z")
                    nc.vector.tensor_add(out=z[:], in0=y_ps[:],
                                         in1=addt[t])
                    nc.vector.tensor_scalar(
                        out=res[:], in0=z[:],
                        scalar1=0.0, scalar2=6.0,
                        op0=Alu.max, op1=Alu.min,
                    )
                    nc.sync.dma_start(out=out_d[:, :], in_=res[:])

    nc.finalize()
    return nc


def _get_program(fast):
    key = ("fast",) if fast else ("general",)
    if key not in _COMPILED:
        if fast:
            _COMPILED[key] = _build_program_fast()
        else:
            import concourse.mybir as mybir
            _COMPILED[key] = _build_program_general(mybir.dt.float32)
    return _COMPILED[key]


def _prep_fast(xs, x_query, decay, learn, learn2, core):
    """Host-side tensor prep for one core's batch slice (fast path)."""
    import ml_dtypes
    g = 1.0 - decay
    bs = slice(core * BPC, (core + 1) * BPC)
    xs_c = xs[:, bs, :]          # [T, 32, M]
    xq_c = x_query[bs, :]        # [32, M]

    # xb: [128, (T+1)*M], query in slot T, replicated over the 4 s4 blocks
    xb = np.concatenate([xs_c, xq_c[None]], axis=0)  # [17, 32, M]
    xb = np.tile(
        xb.transpose(1, 0, 2).reshape(BPC, (T + 1) * M), (4, 1)
    ).astype(ml_dtypes.bfloat16)

    # xhw[k][s4*32+b, m] = learn * g^-(4k+s4+1) * xs[4k+s4, b, m]
    s_idx = np.arange(T, dtype=np.float64)
    wneg = (learn * g ** (-(s_idx + 1.0))).astype(np.float32)
    xhw4 = (xs_c.astype(np.float32) * wneg[:, None, None]).reshape(
        NSTILE, 4, BPC, M
    )
    xhw = xhw4.transpose(1, 2, 0, 3).reshape(128, NSTILE * M)
    xhw = xhw.astype(ml_dtypes.bfloat16)

    # selw[t] = g^t * one-hot(b); partitions (s4, b)
    eye = np.tile(np.eye(BPC, dtype=np.float32), (4, 1))  # [128, 32]
    gpow = (g ** np.arange(T + 1, dtype=np.float64)).astype(np.float32)
    selw = (gpow[:, None, None] * eye[None]).transpose(1, 0, 2).reshape(
        128, (T + 1) * BPC
    ).astype(ml_dtypes.bfloat16)

    eyel2 = (learn2 * np.eye(BPC, dtype=np.float32)).astype(ml_dtypes.bfloat16)

    return {
        "xb": np.ascontiguousarray(xb),
        "xhw": np.ascontiguousarray(xhw),
        "selw": np.ascontiguousarray(selw),
        "eyel2": np.ascontiguousarray(eyel2),
    }


def _prep_general(xs, x_query, A_init, decay, learn, learn2, core):
    import ml_dtypes
    g = 1.0 - decay
    bs = slice(core * BPC, (core + 1) * BPC)
    xs_c = xs[:, bs, :]
    xq_c = x_query[bs, :]
    a_c = A_init[bs]

    xb = np.empty((T + 1, 128, M), dtype=np.float32)
    for t in range(T):
        xb[t] = np.tile(xs_c[t], (4, 1))
    xb[T] = np.tile(xq_c, (4, 1))
    xb = np.ascontiguousarray(xb.transpose(1, 0, 2).reshape(128, (T + 1) * M))

    s_idx = np.arange(T, dtype=np.float64)
    wneg = (learn * g ** (-(s_idx + 1.0))).astype(np.float32)
    xhw4 = (xs_c.astype(np.float32) * wneg[:, None, None]).reshape(
        NSTILE, 4, BPC, M
    )
    xhw = xhw4.transpose(1, 2, 0, 3).reshape(128, NSTILE * M)
    xhw = xhw.astype(ml_dtypes.bfloat16)

    eye = np.tile(np.eye(BPC, dtype=np.float32), (4, 1))
    gpow = (g ** np.arange(T + 1, dtype=np.float64)).astype(np.float32)
    selw = (gpow[:, None, None] * eye[None]).transpose(1, 0, 2).reshape(
        128, (T + 1) * BPC
    ).astype(ml_dtypes.bfloat16)

    q_c = np.einsum("bij,tbj->tbi", a_c, xs_c)
    qq_c = np.einsum("bij,bj->bi", a_c, xq_c)
    addt = np.zeros((T + 1, BPC, M), dtype=np.float32)
    addt[:T] = learn2 * xs_c
    addt[:T] += gpow[:T, None, None] * q_c
    addt[T] = gpow[T] * qq_c
    addt = addt.transpose(1, 0, 2).reshape(BPC, (T + 1) * M)

    return {
        "xb": np.ascontiguousarray(xb),
        "xhw": np.ascontiguousarray(xhw),
        "selw": np.ascontiguousarray(selw),
        "addt": np.ascontiguousarray(addt),
    }


def kernel(A_init, xs, x_query, decay, learn, learn2, _trace=False):
    from concourse.bass_utils import run_bass_kernel_spmd

    xs = np.asarray(xs, dtype=np.float32)
    x_query = np.asarray(x_query, dtype=np.float32)
    A_init = np.asarray(A_init, dtype=np.float32)
    decay_v = float(np.asarray(decay).reshape(-1)[0])
    learn_v = float(np.asarray(learn).reshape(-1)[0])
    learn2_v = float(np.asarray(learn2).reshape(-1)[0])

    # The relu6 -> min(.,6) fold inside the history dots requires provably
    # nonnegative pre-activations: A_init == 0 and all inputs >= 0.
    a_zero = not A_init.any()
    fast = bool(a_zero and xs.min() >= 0.0 and x_query.min() >= 0.0)
    nc = _get_program(fast)

    in_maps = []
    for c in range(NCORES):
        if fast:
            in_maps.append(
                _prep_fast(xs, x_query, decay_v, learn_v, learn2_v, c)
            )
        else:
            in_maps.append(
                _prep_general(xs, x_query, A_init, decay_v, learn_v,
                              learn2_v, c)
            )

    res = run_bass_kernel_spmd(
        nc, in_maps, core_ids=list(range(NCORES)), trace=_trace
    )

    out = np.concatenate(
        [np.asarray(r["out"], dtype=np.float32) for r in res.results], axis=0
    )

    if _trace:
        return out, res
    return out


# revision 17
# speedup vs baseline: 1.0713x; 1.0713x over previous
"""Trainium2 Bass kernel for the Hebbian fast-weight memory module.

Reference computation (B=256 batches, T=16 steps, M=256):
    step t:  p2 = learn * relu6(learn2*x_t + A @ x_t)
             A  = (1-decay)*A + outer(x_t, p2)
    output:  relu6(A_final @ x_query)

Key identity (g = 1-decay, Phi_s = relu6(z_s), z_s = learn2*x_s + y_s):
    y_t[i] = g^t (A_init@x_t)[i] + sum_{s<t} g^{t-1-s} learn (Phi_s . x_t) x_s[i]
    out[i] = relu6(g^16 (A_init@x_q)[i] + sum_s g^{15-s} learn (Phi_s . x_q) x_s[i])
A is never materialized. For the fast path we additionally require
A_init == 0 and xs, x_query >= 0 (checked at runtime): then z >= 0 and
relu6(z) = min(z, 6) folds into the history dot products.

Fast-path layout per core (bpc=32 batches):
  partitions p = s4*32 + b  (s4 in [0,4), b in [0,32)), history step s = 4k+s4
  z-history lives IN PSUM: z_s at PSUM bank (s//4), partitions (s%4)*32+b,
  written directly by the accumulating matmuls (no per-step copy-out).
  Completed history tiles get one lazy ACT copy to SBUF (bf16), 3 total.

  Per step t: for each history tile k: a scalar_tensor_tensor on DVE/Pool
  computes cw_k[p] = sum_m min(z,6)*x_t (relu6 folded); ACT/DVE builds
  selcw_k = g^t*onehot(b)*cw_k; PE accumulates selcw_k.T @ xhw_k into
  z_t's PSUM slot. learn2*x_t enters via a tiny constant matmul (eyeL2).

Sharding: batch 256 -> 8 cores x 32 batches (pure data parallel).
The general path (A_init != 0 or negative inputs) uses the original
baseline program with host-precomputed additive terms.
"""

import os
import sys

for _p in ("/opt/pypackages", "/opt/trn_rl_repo"):
    if _p not in sys.path:
        sys.path.insert(0, _p)

import numpy as np

B, T, M = 256, 16, 256
NCORES = 8
BPC = B // NCORES  # 32 batches per core
NSTILE = 4         # history tiles; each holds 4 steps x 32 batches

_COMPILED = {}


def _dve_old(t, k):
    """True if the old-tile dot (t, k) runs on DVE instead of Pool."""
    return (t * 3 + k) % 4 == 0


def _build_program_fast():
    import concourse.bacc as bacc
    import concourse.mybir as mybir
    from concourse.tile import TileContext

    f32 = mybir.dt.float32
    bf16 = mybir.dt.bfloat16
    Alu = mybir.AluOpType
    Act = mybir.ActivationFunctionType

    nc = bacc.Bacc(target_bir_lowering=False)

    NQ = T + 1  # 16 steps + query
    xb_d = nc.dram_tensor("xb", [128, NQ * M], bf16, kind="ExternalInput")
    xhw_d = nc.dram_tensor("xhw", [128, NSTILE * M], bf16, kind="ExternalInput")
    selw_d = nc.dram_tensor("selw", [128, NQ * BPC], bf16, kind="ExternalInput")
    eyel2_d = nc.dram_tensor("eyel2", [BPC, BPC], bf16, kind="ExternalInput")
    out_d = nc.dram_tensor("out", [BPC, M], f32, kind="ExternalOutput")

    with TileContext(nc) as tc:
        with (
            tc.tile_pool(name="persist", bufs=1) as pp,
            tc.tile_pool(name="work", bufs=28) as wp,
            tc.tile_pool(name="psum", bufs=1, space="PSUM") as psp,
        ):
            xb_all = pp.tile([128, NQ * M], bf16, tag="xb", name="xb_sb")
            xb = [xb_all[:, t * M:(t + 1) * M] for t in range(NQ)]
            xhw_all = pp.tile([128, NSTILE * M], bf16, tag="xhw", name="xhw_sb")
            xhw = [xhw_all[:, k * M:(k + 1) * M] for k in range(NSTILE)]
            selw_all = pp.tile([128, NQ * BPC], bf16, tag="selw", name="selw_sb")
            selw = [selw_all[:, t * BPC:(t + 1) * BPC] for t in range(NQ)]
            eyel2 = pp.tile([BPC, BPC], bf16, tag="eyel2", name="eyel2_sb")
            junk_d = pp.tile([128, M], bf16, tag="junkd", name="junk_dve")
            junk_p = pp.tile([128, M], bf16, tag="junkp", name="junk_pool")
            # z history in PSUM: bank k holds steps 4k..4k+3 (k=4: query y)
            zb = [psp.tile([128, M], f32, tag=f"zb{k}", name=f"zb{k}")
                  for k in range(NSTILE + 1)]
            zjunk = psp.tile([BPC, M], f32, tag="zjunk", name="zjunk")

            # --- input DMAs, split across queues; earliest-needed first ---
            # All input DMAs ride the SP (sync) HWDGE queue: the SP engine
            # is idle so descriptors stream immediately; the gpsimd queue is
            # SWDGE (software-serial) and the scalar engine is busy with
            # selcw activations all kernel.
            XS = 5 * M
            nc.sync.dma_start(out=eyel2[:], in_=eyel2_d[:, :])
            nc.sync.dma_start(out=selw_all[:, :6 * BPC],
                              in_=selw_d[:, :6 * BPC])
            nc.sync.dma_start(out=xb_all[:, :2 * M], in_=xb_d[:, :2 * M])
            nc.scalar.dma_start(out=xhw_all[:], in_=xhw_d[:, :])
            nc.scalar.dma_start(out=selw_all[:, 6 * BPC:],
                                in_=selw_d[:, 6 * BPC:])
            nc.sync.dma_start(out=xb_all[:, 2 * M:XS],
                              in_=xb_d[:, 2 * M:XS])
            nc.sync.dma_start(out=xb_all[:, T * M:], in_=xb_d[:, T * M:])
            nc.sync.dma_start(out=xb_all[:, XS:T * M],
                              in_=xb_d[:, XS:T * M])

            def zslot(s):
                q = s % 4
                return zb[s // 4][q * BPC:(q + 1) * BPC, :]

            def hot_of(t):
                return (t - 1) // 4

            # ---- software-pipelined schedule for old-tile work ----
            # item (tp, k): dot of completed tile k against query x_tp.
            # Legal once ph[k] is copied (window 4k+4) and xb chunk holding
            # tp has landed; must be emitted no later than window tp-1.
            items = []
            for tp in range(2, NQ):
                for k in range(hot_of(tp)):
                    items.append((tp, k))

            def legal_w(tp, k):
                w = 4 * k + 4
                if tp > 4:
                    w = max(w, 3)
                if tp > 9:
                    w = max(w, 5)
                return max(w, 2)

            sched = {w: [] for w in range(1, NQ)}
            loads = {w: 0 for w in range(1, NQ)}
            for tp, k in sorted(items, key=lambda it: legal_w(*it)):
                w = legal_w(tp, k)
                cap = lambda x: 1 if x <= 4 else (2 if x <= 7 else 3)
                late = max(legal_w(tp, k), tp - 2)
                while loads[w] >= cap(w) and w < late:
                    w += 1
                w = min(w, late)
                sched[w].append((tp, k))
                loads[w] += 1

            selcw_of = {}   # (tp, k) -> selcw tile
            started = set()  # PSUM slots whose start mm has been emitted

            def emit_old_dot(tp, k):
                cw = wp.tile([128, 1], f32, tag="cw", name="cw")
                nc.vector.scalar_tensor_tensor(
                    out=junk_d[:], in0=zb[k][:], scalar=6.0, in1=xb[tp],
                    op0=Alu.min, op1=Alu.mult, accum_out=cw[:],
                )
                selcw = wp.tile([128, BPC], bf16, tag="selcw", name="selcw")
                nc.scalar.activation(
                    out=selcw[:], in_=selw[tp][:], func=Act.Copy,
                    scale=cw[:],
                )
                selcw_of[(tp, k)] = selcw

            def dst_of(t):
                return zslot(t) if t < T else zb[NSTILE][0:BPC, :]

            def tpos_of(t):
                return (0, (t % 4) * BPC) if t < T else (0, 0)

            def emit_start_and_old_mms(t):
                # eyeL2 start mm (z = learn2*x_t + ...); query has no
                # learn2 term, so its first old mm carries start=True
                if t < T:
                    nc.tensor.matmul(dst_of(t), eyel2[:], xb[t][0:BPC, :],
                                     start=True, stop=False,
                                     tile_position=tpos_of(t))
                    started.add(t)
                for k in range(hot_of(t)):
                    nc.tensor.matmul(
                        dst_of(t), selcw_of[(t, k)][:], xhw[k],
                        start=(t not in started), stop=False,
                        tile_position=tpos_of(t),
                    )
                    started.add(t)

            # t=0: z_0 = learn2 * x_0
            nc.tensor.matmul(zslot(0), eyel2[:], xb[0][0:BPC, :],
                             start=True, stop=True, tile_position=(0, 0))
            emit_start_and_old_mms(1)

            for t in range(1, NQ):
                is_q = (t == T)
                hot = hot_of(t)
                nrow = ((t - 1) % 4) + 1
                np_ = nrow * BPC

                # --- hot chain: dot from PSUM, selcw on DVE, stop mm ---
                cw = wp.tile([128, 1], f32, tag="cw", name="cw")
                nc.vector.scalar_tensor_tensor(
                    out=junk_d[0:np_, :], in0=zb[hot][0:np_, :], scalar=6.0,
                    in1=xb[t][0:np_, :], op0=Alu.min, op1=Alu.mult,
                    accum_out=cw[0:np_, :],
                )
                selcw = wp.tile([128, BPC], bf16, tag="selcw", name="selcw")
                nc.vector.tensor_scalar(
                    out=selcw[0:np_, :], in0=selw[t][0:np_, :],
                    scalar1=cw[0:np_, :], scalar2=None, op0=Alu.mult,
                )
                nc.tensor.matmul(
                    dst_of(t), selcw[0:np_, :], xhw[hot][0:np_, :],
                    start=(t not in started), stop=True,
                    tile_position=tpos_of(t),
                )

                for tp, k in sched.get(t, []):
                    emit_old_dot(tp, k)
                if t + 1 < NQ:
                    emit_start_and_old_mms(t + 1)

            res = wp.tile([BPC, M], f32, tag="res", name="res")
            nc.vector.tensor_scalar(
                out=res[:], in0=zb[NSTILE][0:BPC, :],
                scalar1=0.0, scalar2=6.0, op0=Alu.max, op1=Alu.min,
            )
            nc.sync.dma_start(out=out_d[:, :], in_=res[:])

    nc.finalize()
    return nc


def _build_program_general(dots_dtype):
    """Baseline program: general path (A_init != 0 or negative inputs)."""
    import concourse.bacc as bacc
    import concourse.mybir as mybir
    from concourse.tile import TileContext

    f32 = mybir.dt.float32
    bf16 = mybir.dt.bfloat16
    Alu = mybir.AluOpType

    nc = bacc.Bacc(target_bir_lowering=False)

    xb_d = nc.dram_tensor("xb", [128, (T + 1) * M], dots_dtype,
                          kind="ExternalInput")
    xhw_d = nc.dram_tensor("xhw", [128, NSTILE * M], bf16, kind="ExternalInput")
    selw_d = nc.dram_tensor("selw", [128, (T + 1) * BPC], bf16,
                            kind="ExternalInput")
    add_d = nc.dram_tensor("addt", [BPC, (T + 1) * M], f32,
                           kind="ExternalInput")
    out_d = nc.dram_tensor("out", [BPC, M], f32, kind="ExternalOutput")

    with TileContext(nc) as tc:
        with (
            tc.tile_pool(name="persist", bufs=1) as pp,
            tc.tile_pool(name="work", bufs=8) as wp,
            tc.tile_pool(name="psum", bufs=6, space="PSUM") as psp,
        ):
            xb_all = pp.tile([128, (T + 1) * M], dots_dtype, tag="xb",
                             name="xb_sb")
            xb = [xb_all[:, t * M:(t + 1) * M] for t in range(T + 1)]
            xhw_all = pp.tile([128, NSTILE * M], bf16, tag="xhw", name="xhw_sb")
            xhw = [xhw_all[:, k * M:(k + 1) * M] for k in range(NSTILE)]
            selw_all = pp.tile([128, (T + 1) * BPC], bf16, tag="selw",
                               name="selw_sb")
            selw = [selw_all[:, t * BPC:(t + 1) * BPC] for t in range(T + 1)]
            addt_all = pp.tile([BPC, (T + 1) * M], f32, tag="addt",
                               name="addt_sb")
            addt = [addt_all[:, t * M:(t + 1) * M] for t in range(T + 1)]
            ph = [pp.tile([128, M], dots_dtype, tag=f"ph{k}", name=f"ph{k}")
                  for k in range(NSTILE)]

            XB_SPLIT = 5 * M
            nc.scalar.dma_start(out=xb_all[:, :XB_SPLIT],
                                in_=xb_d[:, :XB_SPLIT])
            nc.scalar.dma_start(out=addt_all[:], in_=add_d[:, :])
            nc.sync.dma_start(out=selw_all[:], in_=selw_d[:, :])
            nc.sync.dma_start(out=xhw_all[:], in_=xhw_d[:, :])
            nc.sync.dma_start(out=xb_all[:, XB_SPLIT:], in_=xb_d[:, XB_SPLIT:])

            for k in range(NSTILE):
                nc.vector.memset(ph[k][:], 0.0)

            def step_y(t):
                y_ps = psp.tile([BPC, M], f32, tag="y", name="y")
                hot = min(max(t - 1, 0) // 4, NSTILE - 1)
                korder = list(range(hot + 1))
                for i, k in enumerate(korder):
                    junk = wp.tile([128, 1], f32, tag="junk", name="junk")
                    cw = wp.tile([128, 1], f32, tag="cw", name="cw")
                    nc.vector.scalar_tensor_tensor(
                        out=junk.broadcast_to((128, M)),
                        in0=ph[k][:],
                        scalar=1.0,
                        in1=xb[t],
                        op0=Alu.bypass,
                        op1=Alu.mult,
                        accum_out=cw[:],
                    )
                    selcw = wp.tile([128, BPC], bf16, tag="selcw",
                                    name="selcw")
                    nc.vector.tensor_scalar(
                        out=selcw[:], in0=selw[t], scalar1=cw[:],
                        scalar2=None, op0=Alu.mult,
                    )
                    nc.tensor.matmul(
                        y_ps[:], selcw[:], xhw[k],
                        start=(i == 0),
                        stop=(i == len(korder) - 1),
                    )
                return y_ps

            nc.vector.tensor_scalar(
                out=ph[0][0:BPC, :], in0=addt[0],
                scalar1=0.0, scalar2=6.0, op0=Alu.max, op1=Alu.min,
            )
            for t in range(1, T + 1):
                y_ps = step_y(t)
                if t < T:
                    k, s4 = t // 4, t % 4
                    dst = ph[k][s4 * BPC:(s4 + 1) * BPC, :]
                    zt = wp.tile([BPC, M], f32, tag="z", name="z")
                    nc.vector.tensor_add(out=zt[:], in0=y_ps[:],
                                         in1=addt[t])
                    nc.vector.tensor_scalar(
                        out=dst, in0=zt[:],
                        scalar1=0.0, scalar2=6.0,
                        op0=Alu.max, op1=Alu.min,
                    )
                else:
                    res = wp.tile([BPC, M], f32, tag="res", name="res")
                    z = wp.tile([BPC, M], f32, tag="z", name="z")
                    nc.vector.tensor_add(out=z[:], in0=y_ps[:],
                                         in1=addt[t])
                    nc.vector.tensor_scalar(
                        out=res[:], in0=z[:],
                        scalar1=0.0, scalar2=6.0,
                        op0=Alu.max, op1=Alu.min,
                    )
                    nc.sync.dma_start(out=out_d[:, :], in_=res[:])

    nc.finalize()
    return nc


def _get_program(fast):
    key = ("fast",) if fast else ("general",)
    if key not in _COMPILED:
        if fast:
            _COMPILED[key] = _build_program_fast()
        else:
            import concourse.mybir as mybir
            _COMPILED[key] = _build_program_general(mybir.dt.float32)
    return _COMPILED[key]


def _prep_fast(xs, x_query, decay, learn, learn2, core):
    """Host-side tensor prep for one core's batch slice (fast path)."""
    import ml_dtypes
    g = 1.0 - decay
    bs = slice(core * BPC, (core + 1) * BPC)
    xs_c = xs[:, bs, :]          # [T, 32, M]
    xq_c = x_query[bs, :]        # [32, M]

    # xb: [128, (T+1)*M], query in slot T, replicated over the 4 s4 blocks
    xb = np.concatenate([xs_c, xq_c[None]], axis=0)  # [17, 32, M]
    xb = np.tile(
        xb.transpose(1, 0, 2).reshape(BPC, (T + 1) * M), (4, 1)
    ).astype(ml_dtypes.bfloat16)

    # xhw[k][s4*32+b, m] = learn * g^-(4k+s4+1) * xs[4k+s4, b, m]
    s_idx = np.arange(T, dtype=np.float64)
    wneg = (learn * g ** (-(s_idx + 1.0))).astype(np.float32)
    xhw4 = (xs_c.astype(np.float32) * wneg[:, None, None]).reshape(
        NSTILE, 4, BPC, M
    )
    xhw = xhw4.transpose(1, 2, 0, 3).reshape(128, NSTILE * M)
    xhw = xhw.astype(ml_dtypes.bfloat16)

    # selw[t] = g^t * one-hot(b); partitions (s4, b)
    eye = np.tile(np.eye(BPC, dtype=np.float32), (4, 1))  # [128, 32]
    gpow = (g ** np.arange(T + 1, dtype=np.float64)).astype(np.float32)
    selw = (gpow[:, None, None] * eye[None]).transpose(1, 0, 2).reshape(
        128, (T + 1) * BPC
    ).astype(ml_dtypes.bfloat16)

    eyel2 = (learn2 * np.eye(BPC, dtype=np.float32)).astype(ml_dtypes.bfloat16)

    return {
        "xb": np.ascontiguousarray(xb),
        "xhw": np.ascontiguousarray(xhw),
        "selw": np.ascontiguousarray(selw),
        "eyel2": np.ascontiguousarray(eyel2),
    }


def _prep_general(xs, x_query, A_init, decay, learn, learn2, core):
    import ml_dtypes
    g = 1.0 - decay
    bs = slice(core * BPC, (core + 1) * BPC)
    xs_c = xs[:, bs, :]
    xq_c = x_query[bs, :]
    a_c = A_init[bs]

    xb = np.empty((T + 1, 128, M), dtype=np.float32)
    for t in range(T):
        xb[t] = np.tile(xs_c[t], (4, 1))
    xb[T] = np.tile(xq_c, (4, 1))
    xb = np.ascontiguousarray(xb.transpose(1, 0, 2).reshape(128, (T + 1) * M))

    s_idx = np.arange(T, dtype=np.float64)
    wneg = (learn * g ** (-(s_idx + 1.0))).astype(np.float32)
    xhw4 = (xs_c.astype(np.float32) * wneg[:, None, None]).reshape(
        NSTILE, 4, BPC, M
    )
    xhw = xhw4.transpose(1, 2, 0, 3).reshape(128, NSTILE * M)
    xhw = xhw.astype(ml_dtypes.bfloat16)

    eye = np.tile(np.eye(BPC, dtype=np.float32), (4, 1))
    gpow = (g ** np.arange(T + 1, dtype=np.float64)).astype(np.float32)
    selw = (gpow[:, None, None] * eye[None]).transpose(1, 0, 2).reshape(
        128, (T + 1) * BPC
    ).astype(ml_dtypes.bfloat16)

    q_c = np.einsum("bij,tbj->tbi", a_c, xs_c)
    qq_c = np.einsum("bij,bj->bi", a_c, xq_c)
    addt = np.zeros((T + 1, BPC, M), dtype=np.float32)
    addt[:T] = learn2 * xs_c
    addt[:T] += gpow[:T, None, None] * q_c
    addt[T] = gpow[T] * qq_c
    addt = addt.transpose(1, 0, 2).reshape(BPC, (T + 1) * M)

    return {
        "xb": np.ascontiguousarray(xb),
        "xhw": np.ascontiguousarray(xhw),
        "selw": np.ascontiguousarray(selw),
        "addt": np.ascontiguousarray(addt),
    }


def kernel(A_init, xs, x_query, decay, learn, learn2, _trace=False):
    from concourse.bass_utils import run_bass_kernel_spmd

    xs = np.asarray(xs, dtype=np.float32)
    x_query = np.asarray(x_query, dtype=np.float32)
    A_init = np.asarray(A_init, dtype=np.float32)
    decay_v = float(np.asarray(decay).reshape(-1)[0])
    learn_v = float(np.asarray(learn).reshape(-1)[0])
    learn2_v = float(np.asarray(learn2).reshape(-1)[0])

    # The relu6 -> min(.,6) fold inside the history dots requires provably
    # nonnegative pre-activations: A_init == 0 and all inputs >= 0.
    a_zero = not A_init.any()
    fast = bool(a_zero and xs.min() >= 0.0 and x_query.min() >= 0.0)
    nc = _get_program(fast)

    in_maps = []
    for c in range(NCORES):
        if fast:
            in_maps.append(
                _prep_fast(xs, x_query, decay_v, learn_v, learn2_v, c)
            )
        else:
            in_maps.append(
                _prep_general(xs, x_query, A_init, decay_v, learn_v,
                              learn2_v, c)
            )

    res = run_bass_kernel_spmd(
        nc, in_maps, core_ids=list(range(NCORES)), trace=_trace
    )

    out = np.concatenate(
        [np.asarray(r["out"], dtype=np.float32) for r in res.results], axis=0
    )

    if _trace:
        return out, res
    return out
